# revision 16
# baseline (speedup 1.0000x reference)
"""CGCNN-style GNN forward on 8 Trainium2 NeuronCores (Bass/Tile).

Strategy: shard nodes/graphs contiguously across 8 cores (graph-aligned);
edges live on their dst-owner core, sorted by dst into 127-node blocks.
Per layer: per-node projections (PE) -> AllGather of src-projection table
-> edge phase: dma_gather of src rows + one-hot expansion matmul for dst
rows + sigmoid/ln activations + scatter-matmul segment-sum. Host does
index preprocessing only; all FP compute runs on device.
"""
import sys
sys.path.insert(0, '/opt/trn_rl_repo')
import numpy as np
import ml_dtypes

import concourse.bass as bass
import concourse.bacc as bacc
import concourse.mybir as mybir
import concourse.tile as tile
from concourse.bass_utils import run_bass_kernel_spmd
from concourse.library_config import mlp
from concourse.masks import make_identity

bf16 = ml_dtypes.bfloat16
DT = mybir.dt
AF = mybir.ActivationFunctionType
ALU = mybir.AluOpType

NCORES = 8
NL = 5          # graph layers
NFC = 3
H = 128
C = 129
BLKN = 127      # nodes per block (127 + e-row)
CHUNK = 128     # edges per chunk
GRPC = 8        # max chunks per gather call / act batch
NLRUN = NL      # debug: number of graph layers to run
LOHI = 32768    # int16 index split


def _prep(atoms, pos, edge_index, batch, n_graphs):
    n = atoms.shape[0]
    src, dst = edge_index[0].astype(np.int64), edge_index[1].astype(np.int64)
    # graph-aligned node sharding
    gcnt = np.bincount(batch, minlength=n_graphs)
    goff = np.concatenate([[0], np.cumsum(gcnt)])
    targets = [round(c * n / NCORES) for c in range(NCORES + 1)]
    gsplit = [0]
    for c in range(1, NCORES):
        g = int(np.argmin(np.abs(goff - targets[c])))
        gsplit.append(g)
    gsplit.append(n_graphs)
    nod0 = [int(goff[g]) for g in gsplit]  # node offset per core (len 9)
    ncs = [nod0[c + 1] - nod0[c] for c in range(NCORES)]
    NBLK = int(np.ceil(max(ncs) / BLKN))
    NNP = NBLK * BLKN  # padded nodes per core
    GMAX = max(gsplit[c + 1] - gsplit[c] for c in range(NCORES))
    GMAX = ((GMAX + 7) // 8) * 8

    owner = np.searchsorted(np.array(nod0[1:]), dst, side='right')
    cores = []
    for c in range(NCORES):
        m = owner == c
        es, ed = src[m], dst[m] - nod0[c]
        order = np.argsort(ed, kind='stable')
        es, ed = es[order], ed[order]
        blk = ed // BLKN
        bcnt = np.bincount(blk, minlength=NBLK)
        cores.append(dict(es=es, ed=ed, blk=blk, bcnt=bcnt, n_c=ncs[c],
                          g0=gsplit[c], g1=gsplit[c + 1], nod0=nod0[c]))
    # uniform per-block profile: sort blocks per core by edge count desc
    for cd in cores:
        perm = np.argsort(-cd['bcnt'], kind='stable')
        cd['perm'] = perm            # new block i = old block perm[i]
        inv_perm = np.empty(NBLK, np.int64)
        inv_perm[perm] = np.arange(NBLK)
        cd['inv_perm'] = inv_perm    # old block b -> new position
        # node slot map: old local id -> padded slot
        nodemap = -np.ones(NNP, np.int64)   # slot -> old local id
        slot_of = -np.ones(NNP, np.int64)   # old local id (padded idx) -> slot
        for newb in range(NBLK):
            oldb = perm[newb]
            lo_ = oldb * BLKN
            hi_ = min(lo_ + BLKN, cd['n_c'])
            w = hi_ - lo_
            if w > 0:
                nodemap[newb * BLKN: newb * BLKN + w] = np.arange(lo_, hi_)
                slot_of[lo_:hi_] = newb * BLKN + np.arange(w)
        cd['nodemap'] = nodemap
        cd['slot_of'] = slot_of
    # global node id -> allgathered table row
    n = len(atoms_dummy) if False else None
    tabrow = -np.ones(int(nod0[NCORES]), np.int64)
    for c, cd in enumerate(cores):
        loc = np.arange(cd['n_c'])
        tabrow[cd['nod0'] + loc] = c * NNP + cd['slot_of'][loc]
    # per-block lo/hi edge lists in table-row coords
    for cd in cores:
        tr = tabrow[cd['es']]
        newblk = cd['inv_perm'][cd['blk']]
        drel = cd['ed'] - cd['blk'] * BLKN
        blocks = []
        for b in range(NBLK):
            bm = newblk == b
            btr, bdr = tr[bm], drel[bm]
            lo = btr < LOHI
            blocks.append(((btr[lo], bdr[lo]), (btr[~lo] - LOHI, bdr[~lo])))
        cd['blocks'] = blocks
    KLO = [max(int(np.ceil(max(len(cd['blocks'][b][0][0]), 1) / CHUNK)) for cd in cores)
           for b in range(NBLK)]
    KHI = [max(int(np.ceil(max(len(cd['blocks'][b][1][0]), 1) / CHUNK)) for cd in cores)
           for b in range(NBLK)]
    NCH = sum(KLO) + sum(KHI)
    calls = []  # (block, nchunks, is_hi)
    for b in range(NBLK):
        for tot, hi in ((KLO[b], 0), (KHI[b], 1)):
            r = tot
            while r > 0:
                k = min(r, GRPC)
                calls.append((b, k, hi))
                r -= k
    IDXCOLS = NCH * CHUNK // 16

    def pack_idx(flat):  # [n*128] int16 -> [128, n*8] col-major 16-wrap
        a = flat.reshape(-1, 16).T
        return np.tile(a, (8, 1)).astype(np.int16)

    gcnt_all = gcnt
    for cd in cores:
        srcidx = np.zeros(NCH * CHUNK, np.int16)
        dstrel = -np.ones((128, NCH), np.float32)
        ch = 0
        for b in range(NBLK):
            for hi in (0, 1):
                btr, bdr = cd['blocks'][b][hi]
                k = (KLO[b] if hi == 0 else KHI[b])
                nslots = k * CHUNK
                sarr = np.zeros(nslots, np.int16)
                sarr[:len(btr)] = btr.astype(np.int16)
                darr = -np.ones(nslots, np.float32)
                darr[:len(bdr)] = bdr.astype(np.float32)
                srcidx[ch * CHUNK:(ch + k) * CHUNK] = sarr
                dstrel[:, ch:ch + k] = darr.reshape(k, CHUNK).T
                ch += k
        cd['srcidx'] = pack_idx(srcidx)
        cd['dstrel'] = dstrel.astype(bf16)
        nm = cd['nodemap']
        glc = np.where(nm >= 0, nm + cd['nod0'], 0)
        cd['atoms_p'] = atoms[glc].astype(np.int64) * (nm >= 0)
        cd['pos_p'] = pos[glc].astype(np.float32) * (nm >= 0)[:, None]
        Sg = np.zeros((NNP, GMAX), np.float32)
        for i in range(NNP):
            if nm[i] >= 0:
                gid = int(batch[nm[i] + cd['nod0']])
                Sg[i, gid - cd['g0']] = 1.0 / max(gcnt_all[gid], 1)
        cd['Sg'] = Sg
    static = dict(NBLK=NBLK, NNP=NNP, GMAX=GMAX, KLO=KLO, KHI=KHI,
                  NCH=NCH, calls=calls, IDXCOLS=IDXCOLS,
                  NHI=max(1, 50800 - LOHI))
    return cores, static


def _build(st, n_nodes):
    NBLK, NNP, GMAX, NCH = st['NBLK'], st['NNP'], st['GMAX'], st['NCH']
    calls = st['calls']
    NTAB = NNP * NCORES          # allgathered table rows
    NHI = max(NTAB - LOHI, 1)    # hi region rows
    TW = 384                     # table row channels (bf16) = 768B
    EMBCALLS = int(np.ceil(NNP / 1024))
    NGRP = int(np.ceil(NNP / 128))

    nc = bacc.Bacc("TRN2")
    nc.num_devices = NCORES
    f32, b16, i16 = DT.float32, DT.bfloat16, DT.int16

    D = {}
    D['embidx'] = nc.dram_tensor("embidx", [128, EMBCALLS * 64], i16, kind="ExternalInput")
    D['emb'] = nc.dram_tensor("emb", [119, 128], f32, kind="ExternalInput")
    D['posz'] = nc.dram_tensor("posz", [1, NNP], f32, kind="ExternalInput")
    D['pos3'] = nc.dram_tensor("pos3", [NNP, 4], f32, kind="ExternalInput")
    D['idx'] = nc.dram_tensor("idx", [128, st['IDXCOLS']], i16, kind="ExternalInput")
    D['dstrel'] = nc.dram_tensor("dstrel", [128, NCH], b16, kind="ExternalInput")
    D['iota8'] = nc.dram_tensor("iota8", [128, 128], b16, kind="ExternalInput")
    D['Wmain'] = nc.dram_tensor("Wmain", [128, NL * 516], f32, kind="ExternalInput")
    D['Wfixz'] = nc.dram_tensor("Wfixz", [1, NL * 516], f32, kind="ExternalInput")
    D['Wfixb'] = nc.dram_tensor("Wfixb", [1, NL * 516], f32, kind="ExternalInput")
    D['wrow5'] = nc.dram_tensor("wrow5", [NL, 261], b16, kind="ExternalInput")
    D['Sg'] = nc.dram_tensor("Sg", [NNP, GMAX], f32, kind="ExternalInput")
    D['fcm'] = nc.dram_tensor("fcm", [128, NFC * C + 1], f32, kind="ExternalInput")
    D['fcz'] = nc.dram_tensor("fcz", [1, NFC * C + 1], f32, kind="ExternalInput")
    D['fcb'] = nc.dram_tensor("fcb", [1, NFC * C + 1], f32, kind="ExternalInput")
    D['out'] = nc.dram_tensor("out", [GMAX, 1], f32, kind="ExternalOutput")
    D['dbgx'] = nc.dram_tensor("dbgx", [128, NNP], f32, kind="ExternalOutput")
    D['dbgz'] = nc.dram_tensor("dbgz", [1, NNP], f32, kind="ExternalOutput")

    shard_in = nc.dram_tensor("shard_in", [NNP, TW], b16, kind="Internal")
    tabA = nc.dram_tensor("tabA", [NTAB, TW], b16, kind="Internal", addr_space="Shared")
    tabB = nc.dram_tensor("tabB", [NTAB, TW], b16, kind="Internal", addr_space="Shared")
    tabHI = nc.dram_tensor("tabHI", [NHI, TW], b16, kind="Internal")
    ebuf = nc.dram_tensor("ebuf", [128, NCH], f32, kind="Internal")

    with tile.TileContext(nc) as tc:
        with (
            tc.tile_pool(name="const", bufs=1) as cp,
            tc.tile_pool(name="work", bufs=2) as wp,
            tc.tile_pool(name="meta", bufs=3) as mp,
            tc.tile_pool(name="small", bufs=2) as sp,
            tc.tile_pool(name="pproj", bufs=1, space="PSUM") as pj,
            tc.tile_pool(name="pedge", bufs=2, space="PSUM") as pe,
        ):
            nc.gpsimd.load_library(mlp)
            identf = cp.tile([128, 128], f32)
            make_identity(nc, identf[:])
            identb = cp.tile([128, 128], b16)
            nc.vector.tensor_copy(out=identb[:], in_=identf[:])
            iota8 = cp.tile([128, 128], b16)
            nc.sync.dma_start(iota8[:], D['iota8'][:])
            idxs = cp.tile([128, st['IDXCOLS']], i16)
            nc.sync.dma_start(idxs[:], D['idx'][:])
            Wm = cp.tile([128, NL * 516], f32)
            nc.sync.dma_start(Wm[:], D['Wmain'][:])
            Wfz = cp.tile([1, NL * 516], f32)
            nc.sync.dma_start(Wfz[:], D['Wfixz'][:])
            Wfb = cp.tile([1, NL * 516], f32)
            nc.sync.dma_start(Wfb[:], D['Wfixb'][:])
            ones = cp.tile([1, 256], b16)
            nc.vector.memset(ones[:], 1.0)
            onesf = cp.tile([1, 256], f32)
            nc.vector.memset(onesf[:], 1.0)
            xfm = cp.tile([128, NNP], f32)       # x channels 0..127, feat-major
            zrow = cp.tile([1, NNP], f32)        # x channel 128
            astore = cp.tile([128, NBLK * 261], b16)
            wb0 = cp.tile([128, 258], b16)       # wrow layer0 bcast

            # ---- x init: emb gather + transposes ----
            eidx = cp.tile([128, EMBCALLS * 64], i16)
            nc.sync.dma_start(eidx[:], D['embidx'][:])
            nc.sync.dma_start(zrow[:], D['posz'][:])
            for i in range(EMBCALLS):
                embg = wp.tile([128, 8, 128], f32, tag="gt")
                nc.gpsimd.dma_gather(
                    embg[:], D['emb'][:], eidx[:, i * 64:(i + 1) * 64],
                    num_idxs=1024, num_idxs_reg=1024, elem_size=128)
                for g in range(8):
                    gg = i * 8 + g
                    if gg >= NGRP:
                        break
                    w = min(128, NNP - gg * 128)
                    tp = pe.tile([128, 128], f32, tag="tr")
                    nc.tensor.transpose(out=tp[:, :w], in_=embg[:, g, :][:w, :],
                                        identity=identf[0:w, 0:w])
                    nc.vector.tensor_copy(out=xfm[:, gg * 128:gg * 128 + w],
                                          in_=tp[:, :w])
            # astore pos cols + row127 zeros once
            for b in range(NBLK):
                nc.gpsimd.dma_start(
                    out=astore[0:127, b * 261 + 258: b * 261 + 261],
                    in_=D['pos3'][b * BLKN:(b + 1) * BLKN, 0:3])

            zt8 = cp.tile([128, TW], b16)
            nc.vector.memset(zt8[:], 0.0)
            nhw = 0
            while nhw < NHI:
                rr = min(128, NHI - nhw)
                nc.sync.dma_start(tabHI[nhw:nhw + rr, :], zt8[0:rr, :])
                nhw += rr
            nhw = 0
            while nhw < NNP:
                rr = min(128, NNP - nhw)
                nc.sync.dma_start(shard_in[nhw:nhw + rr, 261:384],
                                  zt8[0:rr, 0:123])
                nhw += rr
            wr0 = cp.tile([1, 261], b16)
            nc.sync.dma_start(wr0[:], D['wrow5'][0:1, :])
            wb0p = pe.tile([128, 261], f32, tag="ex")
            nc.tensor.matmul(out=wb0p[:, 0:258], lhsT=ones[0:1, 0:128],
                             rhs=wr0[0:1, 0:258], start=True, stop=True)
            nc.scalar.copy(out=wb0[:], in_=wb0p[:, 0:258])

            for l in range(min(NL, NLRUN)):
                tab = tabA if l % 2 == 0 else tabB
                # ---- projection phase ----
                for b in range(NBLK):
                    psa = pj.tile([BLKN, 258], f32, tag="proja")
                    psb = pj.tile([BLKN, 258], f32, tag="projb")
                    xt = xfm[:, b * BLKN:(b + 1) * BLKN]
                    zt = zrow[0:1, b * BLKN:(b + 1) * BLKN]
                    for h, o in enumerate((psa, psb)):
                        cs = l * 516 + h * 258
                        nc.tensor.matmul(out=o[:], lhsT=xt, rhs=Wm[:, cs:cs + 258],
                                         start=True, stop=False)
                        nc.tensor.matmul(out=o[:], lhsT=zt, rhs=Wfz[:, cs:cs + 258],
                                         start=False, stop=False)
                        nc.tensor.matmul(out=o[:], lhsT=onesf[0:1, 0:BLKN],
                                         rhs=Wfb[:, cs:cs + 258],
                                         start=False, stop=True)
                    nc.scalar.copy(out=astore[0:127, b * 261:b * 261 + 258],
                                   in_=psa[:])
                    tst = sp.tile([BLKN, 258], b16, tag="tst")
                    nc.scalar.copy(out=tst[:], in_=psb[:])
                    nc.sync.dma_start(shard_in[b * BLKN:(b + 1) * BLKN, 0:258],
                                      tst[:])
                    nc.sync.dma_start(
                        astore[127:128, b * 261:(b + 1) * 261],
                        D['wrow5'][l:l + 1, :])
                # layer-0 shard pos cols
                if l == 0:
                    for b in range(NBLK):
                        nc.gpsimd.dma_start(
                            out=shard_in[b * BLKN:(b + 1) * BLKN, 258:261],
                            in_=D['pos3'][b * BLKN:(b + 1) * BLKN, 0:3])
                nc.gpsimd.collective_compute(
                    "AllGather", ALU.bypass,
                    replica_groups=[list(range(NCORES))],
                    ins=[shard_in[:]], outs=[tab[:]])
                if NTAB > LOHI:
                    nc.sync.dma_start(tabHI[:], tab[LOHI:LOHI + NHI, :])

                # ---- edge phase ----
                ch = 0
                co = 0
                cb = {b: 0 for b in range(NBLK)}
                accs_by_b = {}
                for (b, k, hi) in calls:
                    if cb[b] == 0:
                        acc_t = pe.tile([128, C], f32, tag="acc")
                        accs_by_b[b] = acc_t
                    acc = accs_by_b[b]
                    nb = st['KLO'][b] + st['KHI'][b]
                    gt = wp.tile([128, GRPC, TW], b16, tag="gt")
                    srct = tabHI if hi else tab
                    nc.gpsimd.dma_gather(
                        gt[:, :k, :], srct[:], idxs[:, co:co + k * 8],
                        num_idxs=k * 128, num_idxs_reg=k * 128, elem_size=TW)
                    dr = mp.tile([128, GRPC], b16, tag="dr")
                    nc.sync.dma_start(dr[:, :k], D['dstrel'][:, ch:ch + k])
                    sel8 = wp.tile([128, GRPC * 128], b16, tag="sel8")
                    for j in range(k):
                        nc.vector.tensor_tensor(
                            out=sel8[:, j * 128:(j + 1) * 128],
                            in0=dr[:, j:j + 1].to_broadcast([128, 128]),
                            in1=iota8[:], op=ALU.is_equal)
                    if l > 0:
                        ev = mp.tile([128, GRPC], f32, tag="ev")
                        nc.sync.dma_start(ev[:, :k], ebuf[:, ch:ch + k])
                        for j in range(k):
                            nc.vector.tensor_copy(
                                out=sel8[:, j * 128 + 127: j * 128 + 128],
                                in_=ev[:, j:j + 1])
                    pre = wp.tile([128, GRPC, 258], b16, tag="pre")
                    if l == 0:
                        pd8 = mp.tile([128, GRPC * 3], b16, tag="pd8")
                    for j in range(k):
                        selT = sp.tile([128, 128], b16, tag="selT")
                        nc.sync.dma_start(selT[:], sel8[:, j * 128:(j + 1) * 128],
                                          transpose=True)
                        ex = pe.tile([128, 261], f32, tag="ex")
                        nc.tensor.matmul(
                            out=ex[:], lhsT=selT[:],
                            rhs=astore[:, b * 261:(b + 1) * 261],
                            start=True, stop=True)
                        nc.vector.tensor_add(out=pre[:, j, :], in0=ex[:, 0:258],
                                             in1=gt[:, j, 0:258])
                        if l == 0:
                            nc.vector.tensor_tensor(
                                out=pd8[:, j * 3:(j + 1) * 3],
                                in0=gt[:, j, 258:261], in1=ex[:, 258:261],
                                op=ALU.subtract)
                    if l == 0:
                        sq = mp.tile([128, GRPC * 3], b16, tag="sq")
                        nc.vector.tensor_mul(out=sq[:, :k * 3], in0=pd8[:, :k * 3],
                                             in1=pd8[:, :k * 3])
                        e2 = mp.tile([128, GRPC], f32, tag="e2")
                        nc.vector.tensor_reduce(
                            out=e2[:, :k],
                            in_=sq[:, :k * 3].rearrange("p (k t) -> p k t", t=3),
                            axis=mybir.AxisListType.X, op=ALU.add)
                        e8 = mp.tile([128, GRPC], f32, tag="e8")
                        nc.scalar.activation(out=e8[:, :k], in_=e2[:, :k],
                                             func=AF.Sqrt)
                        nc.sync.dma_start(ebuf[:, ch:ch + k], e8[:, :k])
                        for j in range(k):
                            ew = sp.tile([128, 258], b16, tag="ew")
                            nc.vector.tensor_scalar_mul(
                                out=ew[:], in0=wb0[:], scalar1=e8[:, j:j + 1])
                            nc.vector.tensor_add(out=pre[:, j, :],
                                                 in0=pre[:, j, :], in1=ew[:])
                    gate = wp.tile([128, GRPC, C], f32, tag="gate")
                    sarg = wp.tile([128, GRPC, C], f32, tag="sarg")
                    rlu = wp.tile([128, GRPC, C], f32, tag="rlu")
                    nc.scalar.activation(out=gate[:, :k, :], in_=pre[:, :k, 0:129],
                                         func=AF.Sigmoid)
                    nc.scalar.activation(out=rlu[:, :k, :], in_=pre[:, :k, 129:258],
                                         func=AF.Relu)
                    nc.scalar.activation(out=sarg[:, :k, :], in_=pre[:, :k, 129:258],
                                         func=AF.Abs)
                    nc.scalar.activation(out=sarg[:, :k, :], in_=sarg[:, :k, :],
                                         func=AF.Sigmoid)
                    nc.scalar.activation(out=sarg[:, :k, :], in_=sarg[:, :k, :],
                                         func=AF.Ln)
                    nc.vector.tensor_tensor(out=sarg[:, :k, :], in0=sarg[:, :k, :],
                                            in1=rlu[:, :k, :], op=ALU.subtract)
                    m8 = wp.tile([128, GRPC, C], b16, tag="m8")
                    nc.vector.tensor_mul(out=m8[:, :k, :], in0=gate[:, :k, :],
                                         in1=sarg[:, :k, :])
                    for j in range(k):
                        nc.tensor.matmul(
                            out=acc[:], lhsT=sel8[:, j * 128:(j + 1) * 128],
                            rhs=m8[:, j, :],
                            start=(cb[b] + j == 0), stop=(cb[b] + j == nb - 1))
                    cb[b] += k
                    ch += k
                    co += k * 8
                    if cb[b] == nb:
                        accs = sp.tile([128, C], f32, tag="accs")
                        nc.vector.tensor_copy(out=accs[:], in_=acc[:])
                        trm = pe.tile([128, 128], f32, tag="tr")
                        nc.tensor.transpose(out=trm[:, 0:BLKN],
                                            in_=accs[0:BLKN, 0:128],
                                            identity=identf[0:BLKN, 0:BLKN])
                        xs = xfm[:, b * BLKN:(b + 1) * BLKN]
                        nc.vector.tensor_tensor(out=xs, in0=xs,
                                                in1=trm[:, 0:BLKN],
                                                op=ALU.subtract)
                        trz = pe.tile([1, 128], f32, tag="tr")
                        nc.tensor.transpose(out=trz[0:1, 0:BLKN],
                                            in_=accs[0:BLKN, 128:129],
                                            identity=identf[0:BLKN, 0:BLKN])
                        zs = zrow[0:1, b * BLKN:(b + 1) * BLKN]
                        nc.vector.tensor_tensor(out=zs, in0=zs,
                                                in1=trz[:, 0:BLKN],
                                                op=ALU.subtract)

            nc.sync.dma_start(D['dbgx'][:], xfm[:])
            nc.sync.dma_start(D['dbgz'][:], zrow[:])
            # ---- pooling + FC head ----
            fcm = cp.tile([128, NFC * C + 1], f32)
            nc.sync.dma_start(fcm[:], D['fcm'][:])
            fcz = cp.tile([1, NFC * C + 1], f32)
            nc.sync.dma_start(fcz[:], D['fcz'][:])
            fcb = cp.tile([1, NFC * C + 1], f32)
            nc.sync.dma_start(fcb[:], D['fcb'][:])
            gp = pj.tile([GMAX, C], f32, tag="proja")
            for b in range(NBLK):
                sgt = sp.tile([BLKN, GMAX], f32, tag="sgt")
                nc.sync.dma_start(sgt[:], D['Sg'][b * BLKN:(b + 1) * BLKN, :])
                xt = pe.tile([128, 128], f32, tag="tr")
                nc.tensor.transpose(out=xt[0:BLKN, 0:128],
                                    in_=xfm[:, b * BLKN:(b + 1) * BLKN],
                                    identity=identf[:])
                xnm = sp.tile([BLKN, C], f32, tag="xnm")
                nc.vector.tensor_copy(out=xnm[:, 0:128], in_=xt[0:BLKN, 0:128])
                zc = pe.tile([128, 1], f32, tag="tr")
                nc.tensor.transpose(out=zc[0:BLKN, 0:1],
                                    in_=zrow[0:1, b * BLKN:(b + 1) * BLKN],
                                    identity=identf[0:1, 0:1])
                nc.vector.tensor_copy(out=xnm[:, 128:129], in_=zc[0:BLKN, 0:1])
                nc.tensor.matmul(out=gp[:], lhsT=sgt[:], rhs=xnm[:],
                                 start=(b == 0), stop=(b == NBLK - 1))
            g = sp.tile([GMAX, C], f32, tag="g")
            nc.vector.tensor_copy(out=g[:], in_=gp[:])
            for fc in range(NFC + 1):
                w0 = fc * C
                nw = C if fc < NFC else 1
                gfmp = pe.tile([128, GMAX], f32, tag="tr")
                nc.tensor.transpose(out=gfmp[:, 0:GMAX], in_=g[:, 0:128],
                                    identity=identf[0:GMAX, 0:GMAX])
                gfm = sp.tile([128, GMAX], f32, tag="gfm")
                nc.vector.tensor_copy(out=gfm[:], in_=gfmp[:, 0:GMAX])
                gzp = pe.tile([1, 128], f32, tag="tr")
                nc.tensor.transpose(out=gzp[0:1, 0:GMAX], in_=g[:, 128:129],
                                    identity=identf[0:GMAX, 0:GMAX])
                gz = sp.tile([1, GMAX], f32, tag="gz")
                nc.vector.tensor_copy(out=gz[:], in_=gzp[:, 0:GMAX])
                gn = pe.tile([GMAX, C], f32, tag="ex")
                nc.tensor.matmul(out=gn[:, 0:nw], lhsT=gfm[:],
                                 rhs=fcm[:, w0:w0 + nw], start=True, stop=False)
                nc.tensor.matmul(out=gn[:, 0:nw], lhsT=gz[:],
                                 rhs=fcz[:, w0:w0 + nw], start=False, stop=False)
                nc.tensor.matmul(out=gn[:, 0:nw], lhsT=onesf[0:1, 0:GMAX],
                                 rhs=fcb[:, w0:w0 + nw], start=False, stop=True)
                g2 = sp.tile([GMAX, C], f32, tag="g")
                nc.vector.tensor_copy(out=g2[:, 0:nw], in_=gn[:, 0:nw])
                g = g2
            nc.sync.dma_start(D['out'][:], g[:, 0:1])
    nc.compile()
    return nc, D


def _host_inputs(cores, st, inputs):
    NL_, C_ = NL, C
    Wf, Ws = np.asarray(inputs['Wf']), np.asarray(inputs['Ws'])
    bf_, bs_ = np.asarray(inputs['bf']), np.asarray(inputs['bs'])
    emb = np.asarray(inputs['emb'], np.float32)
    NNP, GMAX, NCH = st['NNP'], st['GMAX'], st['NCH']
    EMBCALLS = int(np.ceil(NNP / 1024))

    Wall = np.zeros((NL_, 129, 516), np.float32)
    for l in range(NL_):
        Wall[l, :, 0:129] = Wf[l][0:129]
        Wall[l, :, 129:258] = Ws[l][0:129]
        Wall[l, :, 258:387] = Wf[l][129:258]
        Wall[l, :, 387:516] = Ws[l][129:258]
    Wmain = np.concatenate([Wall[l, 0:128, :] for l in range(NL_)], axis=1)
    Wfixz = np.concatenate([Wall[l, 128:129, :] for l in range(NL_)], axis=1)
    bias = np.zeros((NL_, 1, 516), np.float32)
    for l in range(NL_):
        bias[l, 0, 0:129] = bf_[l]
        bias[l, 0, 129:258] = bs_[l]
    Wfixb = np.concatenate([bias[l] for l in range(NL_)], axis=1)
    wrow5 = np.zeros((NL_, 261), np.float32)
    for l in range(NL_):
        wrow5[l, 0:129] = Wf[l][258]
        wrow5[l, 129:258] = Ws[l][258]
    iota8 = np.tile(np.arange(128, dtype=np.float32), (128, 1))
    iota8[:, 127] = -1.0
    Wfc = np.asarray(inputs['Wfc'], np.float32)
    bfc = np.asarray(inputs['bfc'], np.float32)
    Wout = np.asarray(inputs['Wout'], np.float32)
    bout = np.asarray(inputs['bout'], np.float32)
    fcm = np.zeros((128, NFC * C_ + 1), np.float32)
    fcz = np.zeros((1, NFC * C_ + 1), np.float32)
    fcb = np.zeros((1, NFC * C_ + 1), np.float32)
    for fc in range(NFC):
        fcm[:, fc * C_:(fc + 1) * C_] = Wfc[fc][0:128]
        fcz[0, fc * C_:(fc + 1) * C_] = Wfc[fc][128]
        fcb[0, fc * C_:(fc + 1) * C_] = bfc[fc]
    fcm[:, NFC * C_] = Wout[0:128, 0]
    fcz[0, NFC * C_] = Wout[128, 0]
    fcb[0, NFC * C_] = bout[0]

    maps = []
    for cd in cores:
        ei = np.zeros(EMBCALLS * 1024, np.int64)
        ei[:NNP] = cd['atoms_p']
        ei16 = np.tile(ei.astype(np.int16).reshape(-1, 16).T, (8, 1))
        pos4 = np.zeros((NNP, 4), np.float32)
        pos4[:, 0:3] = cd['pos_p']
        maps.append({
            "embidx": ei16, "emb": emb,
            "posz": cd['pos_p'][:, 2].reshape(1, -1).astype(np.float32),
            "pos3": pos4,
            "idx": cd['srcidx'], "dstrel": cd['dstrel'],
            "iota8": iota8.astype(bf16),
            "Wmain": Wmain, "Wfixz": Wfixz,
            "Wfixb": Wfixb, "wrow5": wrow5.astype(bf16),
            "Sg": cd['Sg'], "fcm": fcm, "fcz": fcz, "fcb": fcb,
        })
    return maps


def kernel(**inputs):
    atoms = np.asarray(inputs['atoms'], np.int64)
    pos = np.asarray(inputs['pos'], np.float32)
    edge_index = np.asarray(inputs['edge_index'], np.int64)
    batch = np.asarray(inputs['batch'], np.int64)
    n_graphs = 256
    cores, st = _prep(atoms, pos, edge_index, batch, n_graphs)
    nc, D = _build(st, atoms.shape[0])
    maps = _host_inputs(cores, st, inputs)
    res = run_bass_kernel_spmd(nc, maps, core_ids=list(range(NCORES)))
    out = np.zeros((n_graphs, 1), np.float32)
    for c, cd in enumerate(cores):
        g0, g1 = cd['g0'], cd['g1']
        out[g0:g1, 0] = res.results[c]["out"][:g1 - g0, 0]
    return out
